# revision 32
# baseline (speedup 1.0000x reference)
"""GAT additive-attention kernel (nn_GAT) for 8 Trainium2 NeuronCores.

reference:
    k = x @ w_k; q = x @ w_q                      # [bz, N, 1]
    s[b,i,j]   = leaky_relu(k[b,i] + q[b,j], 0.2)
    attn       = softmax(s, axis=j)
    out        = (attn @ x).transpose(0, 2, 1)    # [bz, F, N]

Key identity: with sigma[i,j] = sign(k_i + q_j),
    exp(lrelu(s)) = exp(k_i)exp(q_j)       if s > 0
                  = exp(.2 k_i)exp(.2 q_j) if s <= 0
so with U = exp(q_j - qm)*[x_j|1], V = exp(.2(q_j - qm))*[x_j|1]:
    out_i = num_i / den_i,  [num|den]_i = (M@U)_i + e_i * (colsum(V) - (M@V))_i
where M = (sigma+1)/2 and e_i = exp(-.8 (k_i + qm)).  Everything reduces
to ONE N x N masked matmul  T = sigma-ish @ [U|V]  (fp16, rescaled by the
global qm = max q so fp16 never overflows; absmax error ~3e-4) plus
O(N*F) pre/post work.  No N^2 exp, no N^2 softmax reductions, no
N^2 transposes.

All 16 row-chunk accumulators live in PSUM simultaneously by packing
2-3 accumulation groups per bank: start=True (which clears the WHOLE
bank's has_written bits) is only used by the first group touching each
bank; later groups open with start=False, which overwrites where the
bits are clear and accumulates afterwards.  One mask per j-chunk then
feeds all 16 accumulators, and the per-column sums (gamma/delta, for
the mask-convention correction and the V column-sum) ride along as one
extra matmul per j-chunk into the 8th bank.

Sharding: core c handles batch b = c//2, row-half h = c%2 (2048 rows),
fully data-parallel (no collectives).  The host pre-packs inputs so each
is a single contiguous DMA, and re-packs the [128,16,64] outputs.
"""

import sys
import numpy as np

for _p in ("/opt/trn_rl_repo",):
    if _p not in sys.path:
        sys.path.insert(0, _p)

N = 4096
F = 64
BZ = 4
HALF = 2048
NCH = 32          # j-chunks of 128 (full N)
NIC = 16          # i-chunks of 128 (this core's half)
NACT = 12         # number of j-chunks whose masks run on ScalarE (Sign)
# interleave ACT/DVE mask chunks so both engines produce in parallel
ACTSET = frozenset(c for c in range(NCH) if c % 16 < 7)
NEG_SLOPE = 0.2
SLOT_OFF = 176    # fp32 elems between accumulation groups within a bank

_CACHE = {}


def _body(nc, tc):
    import dataclasses
    import concourse.mybir as mybir
    from concourse import bass_isa

    f32 = mybir.dt.float32
    f16 = mybir.dt.float16
    bf16 = mybir.dt.bfloat16
    Alu = mybir.AluOpType
    Act = mybir.ActivationFunctionType
    Ax = mybir.AxisListType

    # host-prepacked inputs (see make_in_maps)
    xfp_d = nc.dram_tensor("xfp", [128, NCH * F], f32, kind="ExternalInput").ap()
    xhp_d = nc.dram_tensor("xhp", [128, NIC * F], f32, kind="ExternalInput").ap()
    xht_d = nc.dram_tensor("xht", [64, HALF], bf16, kind="ExternalInput").ap()
    wqkr_d = nc.dram_tensor("wqkr", [1, 128], f32, kind="ExternalInput").ap()
    wqkf_d = nc.dram_tensor("wqkf", [64, 2], bf16, kind="ExternalInput").ap()
    out_d = nc.dram_tensor("out", [128, NIC * F], f32, kind="ExternalOutput").ap()

    with (
        tc.tile_pool(name="const", bufs=1) as cp,
        tc.tile_pool(name="sb", bufs=1) as sp,
        tc.tile_pool(name="maskp", bufs=8) as mp,
    ):
        ones_row = cp.tile([1, 128], f32)
        nc.gpsimd.memset(ones_row[:], 1.0)
        ones_col16 = cp.tile([128, 1], f16)
        nc.gpsimd.memset(ones_col16[:], 1.0)
        ones_rowb = cp.tile([1, 128], bf16)
        nc.gpsimd.memset(ones_rowb[:], 1.0)

        # ---- load inputs ----
        xht = sp.tile([64, HALF], bf16)
        nc.sync.dma_start(out=xht[:], in_=xht_d[:])
        wqkf = sp.tile([64, 2], bf16)
        nc.sync.dma_start(out=wqkf[:], in_=wqkf_d[:])
        wqkr = sp.tile([1, 128], f32)
        nc.sync.dma_start(out=wqkr[:], in_=wqkr_d[:])
        xfp = sp.tile([128, NCH, F], f32)
        for g in range(4):  # quarters, so q-building starts early
            nc.sync.dma_start(
                out=xfp[:, g * 8:(g + 1) * 8, :].rearrange("p c f -> p (c f)"),
                in_=xfp_d[:, g * 8 * F:(g + 1) * 8 * F])
        xhp = sp.tile([128, NIC, F], f32)
        for g in range(2):
            nc.gpsimd.dma_start(
                out=xhp[:, g * 8:(g + 1) * 8, :].rearrange("p c f -> p (c f)"),
                in_=xhp_d[:, g * 8 * F:(g + 1) * 8 * F])

        # ---- persistent sbuf ----
        q = sp.tile([128, NCH], f32)
        negq = sp.tile([128, NCH], f32)
        k = sp.tile([128, NIC], f32)
        eq = sp.tile([128, NCH], f32)
        eq2 = sp.tile([128, NCH], f32)
        e = sp.tile([128, NIC], f32)
        ktrow = sp.tile([1, HALF], bf16)
        k_bcast = sp.tile([128, HALF], f16)
        W = sp.tile([128, NCH, 130], f16)   # [U|u|V|v] * exp(-qm) scaling
        S_all = sp.tile([128, NIC, 130], f32)
        C_all = sp.tile([128, NIC, 65], f32)
        rr = sp.tile([128, NIC], f32)
        o_sb = sp.tile([128, NIC, F], f32)
        gb_sb = sp.tile([128, 130], f32)
        gam = sp.tile([1, 130], f32)
        dlt = sp.tile([1, 130], f32)
        G_sb = sp.tile([1, 130], f32)
        wqkb = sp.tile([128, 128], f32)
        tmpq = sp.tile([128, NCH, F], f32)
        junk = sp.tile([128, F], f32)
        qm_neg = sp.tile([128, 1], f32)
        eb2 = sp.tile([128, 1], f32)
        eb8 = sp.tile([128, 1], f32)
        qm1 = sp.tile([128, 1], f32)
        qrow = sp.tile([128, 1], f32)

        with tc.tile_pool(name="pre_ps", bufs=1, space="PSUM") as pp:
            # wq/wk broadcast to all partitions
            wqkb_ps = pp.tile([128, 128], f32)
            nc.tensor.matmul(wqkb_ps[:], ones_row[:], wqkr[:], start=True, stop=True)
            nc.scalar.copy(wqkb[:], wqkb_ps[:])

            # q[p,c] = sum_f x[c*128+p, f] * wq[f]  (big ops, one per xfp quarter)
            wqb = wqkb[:, 0:64]
            wqb3 = dataclasses.replace(wqb, ap=[wqb.ap[0], [0, 8], wqb.ap[1]])
            for g in range(4):
                cs = slice(g * 8, (g + 1) * 8)
                nc.vector.tensor_tensor(tmpq[:, cs, :], xfp[:, cs, :], wqb3, Alu.mult)
                nc.vector.tensor_reduce(q[:, cs], tmpq[:, cs, :], Ax.X, Alu.add)
            nc.vector.tensor_scalar(negq[:], q[:], -1.0, None, Alu.mult)

            # k as a single row ordered by i, then broadcast down partitions
            krps = pp.tile([128, 4, 512], f32, tag="big4")
            for g in range(4):
                nc.tensor.matmul(krps[0:1, g, :], wqkf[:, 1:2],
                                 xht[:, g * 512:(g + 1) * 512], start=True, stop=True)
                nc.scalar.copy(ktrow[0:1, g * 512:(g + 1) * 512], krps[0:1, g, :])
            kbp = pp.tile([128, 4, 512], f32, tag="big4")
            for g in range(4):
                nc.tensor.matmul(kbp[:, g, :], ones_rowb[:],
                                 ktrow[0:1, g * 512:(g + 1) * 512], start=True, stop=True)
                nc.scalar.copy(k_bcast[:, g * 512:(g + 1) * 512], kbp[:, g, :])

            # qm = max(q); shifted exponentials (fp16-safe)
            nc.vector.tensor_reduce(qrow[:], q[:], Ax.X, Alu.max)
            nc.gpsimd.partition_all_reduce(qm1[:], qrow[:], channels=128,
                                           reduce_op=bass_isa.ReduceOp.max)
            nc.vector.tensor_scalar(qm_neg[:], qm1[:], -1.0, None, Alu.mult)
            nc.vector.tensor_scalar(eb2[:], qm_neg[:], 0.2, None, Alu.mult)
            nc.scalar.activation(eq[:], q[:], Act.Exp, bias=qm_neg[:])
            nc.scalar.activation(eq2[:], q[:], Act.Exp, bias=eb2[:], scale=0.2)

            # ---- W build (fp16; in pieces of 8 chunks so matmuls start early) ----
            for g in range(4):
                cs = slice(g * 8, (g + 1) * 8)
                eq_s = eq[:, cs]
                eq2_s = eq2[:, cs]
                eq_b = dataclasses.replace(eq_s, ap=[eq_s.ap[0], eq_s.ap[1], [0, F]])
                eq2_b = dataclasses.replace(eq2_s, ap=[eq2_s.ap[0], eq2_s.ap[1], [0, F]])
                nc.vector.tensor_tensor(W[:, cs, 0:64], xfp[:, cs, :], eq_b, Alu.mult)
                nc.vector.tensor_copy(W[:, cs, 64:65], eq_s)
                nc.vector.tensor_tensor(W[:, cs, 65:129], xfp[:, cs, :], eq2_b, Alu.mult)
                nc.vector.tensor_copy(W[:, cs, 129:130], eq2_s)

        # ---- main masked matmuls: single pass, 16 accumulators in 7 banks ----
        # i-chunk ic -> bank ic % 7, column offset (ic // 7) * SLOT_OFF.
        # gamma/delta colsums ride in the 8th bank (offsets 0 / SLOT_OFF).
        with tc.tile_pool(name="mmps", bufs=1, space="PSUM") as mps:
            mm = mps.tile([128, 7, 512], f32)
            gps = mps.tile([1, 512], f32)
            for c in range(NCH):
                m = mp.tile([128, HALF], f16, tag="mask")
                if c in ACTSET:
                    nc.scalar.activation(m[:], k_bcast[:], Act.Sign, bias=q[:, c:c + 1])
                else:
                    nc.vector.tensor_scalar(m[:], k_bcast[:],
                                            negq[:, c:c + 1], 2.0, Alu.is_gt, Alu.mult)
                for ic in range(NIC):
                    bank, slot = ic % 7, ic // 7
                    off = slot * SLOT_OFF
                    nc.tensor.matmul(mm[:, bank, off:off + 130],
                                     m[:, ic * 128:(ic + 1) * 128],
                                     W[:, c, :],
                                     start=(c == 0 and slot == 0),
                                     stop=(c == NCH - 1),
                                     skip_group_check=True)
                if c == NCH // 2:  # k dot-products fill DVE bubbles late in the loop
                    wkb = wqkb[:, 64:128]
                    wkb3 = dataclasses.replace(wkb, ap=[wkb.ap[0], [0, NIC], wkb.ap[1]])
                    nc.vector.tensor_tensor(tmpq[:, 0:NIC, :], xhp[:], wkb3, Alu.mult)
                    nc.vector.tensor_reduce(k[:], tmpq[:, 0:NIC, :], Ax.X, Alu.add)
                goff = 0 if c in ACTSET else SLOT_OFF
                last_in_set = (c == max(x for x in range(NCH)
                                        if (x in ACTSET) == (c in ACTSET)))
                nc.tensor.matmul(gps[0:1, goff:goff + 130], ones_col16[:], W[:, c, :],
                                 start=(c == 0), stop=last_in_set,
                                 skip_group_check=True)
            # evacuate: S_all[ic] <- mm[bank(ic), off(ic)]
            nc.scalar.copy(S_all[:, 0:7, :], mm[:, :, 0:130])
            nc.scalar.copy(S_all[:, 7:14, :], mm[:, :, SLOT_OFF:SLOT_OFF + 130])
            nc.scalar.copy(S_all[:, 14:16, :], mm[:, 0:2, 2 * SLOT_OFF:2 * SLOT_OFF + 130])
            nc.vector.tensor_scalar(eb8[:], qm_neg[:], 0.8, None, Alu.mult)
            nc.scalar.activation(e[:], k[:], Act.Exp, bias=eb8[:], scale=-0.8)
            nc.vector.tensor_copy(gam[:], gps[0:1, 0:130])
            nc.vector.tensor_copy(dlt[:], gps[0:1, SLOT_OFF:SLOT_OFF + 130])

        # ---- G correction, broadcast, post ----
        # G = [gamma_U | gamma_V - 2*Fv],  Fv = gamma_V + delta_V
        #   so G_V = -gamma_V - 2*delta_V = (dlt_V * -2) - gam_V
        with tc.tile_pool(name="cps", bufs=1, space="PSUM") as cpp:
            nc.vector.tensor_copy(G_sb[:, 0:65], gam[:, 0:65])
            nc.vector.scalar_tensor_tensor(
                G_sb[:, 65:130], dlt[:, 65:130], -2.0, gam[:, 65:130],
                Alu.mult, Alu.subtract)
            gbp = cpp.tile([128, 130], f32)
            nc.tensor.matmul(gbp[:], ones_row[:], G_sb[:], start=True, stop=True)
            nc.vector.tensor_copy(gb_sb[:], gbp[:])

        # S += G ; C = e*S_V - S_U ; rr = 1/C[:,64] ; out = C[:, :64]*rr
        for lo, hi in ((0, 7), (7, 14), (14, 16)):
            nsl = hi - lo
            gb_b = dataclasses.replace(
                gb_sb[:], ap=[gb_sb[:].ap[0], [0, nsl], gb_sb[:].ap[1]])
            sl = S_all[:, lo:hi, :]
            nc.vector.tensor_tensor(sl, sl, gb_b, Alu.add)
            for ic in range(lo, hi):
                nc.vector.scalar_tensor_tensor(
                    C_all[:, ic, :], S_all[:, ic, 65:130], e[:, ic:ic + 1],
                    S_all[:, ic, 0:65], Alu.mult, Alu.subtract)
            nc.vector.reciprocal(rr[:, lo:hi], C_all[:, lo:hi, 64:65])
            for ic in range(lo, hi):
                nc.vector.tensor_scalar(o_sb[:, ic, :], C_all[:, ic, 0:64],
                                        rr[:, ic:ic + 1], None, Alu.mult)
            nc.sync.dma_start(out=out_d[:, lo * F:hi * F], in_=o_sb[:, lo:hi, :])


def build_program():
    if "nc" in _CACHE:
        return _CACHE["nc"]
    from concourse import bacc, tile

    nc = bacc.Bacc("TRN2", target_bir_lowering=False, debug=False,
                   enable_asserts=True, num_devices=8)
    with tile.TileContext(nc) as tc:
        _body(nc, tc)
    nc.compile()
    _CACHE["nc"] = nc
    return nc


def make_in_maps(x, weight_key, weight_query):
    x = np.ascontiguousarray(np.asarray(x, dtype=np.float32))
    wk = np.asarray(weight_key, dtype=np.float32).reshape(-1)
    wq = np.asarray(weight_query, dtype=np.float32).reshape(-1)
    import ml_dtypes
    wqkr = np.concatenate([wq, wk])[None, :].astype(np.float32)      # [1, 128]
    wqkf = np.ascontiguousarray(
        np.stack([wq, wk], axis=1).astype(ml_dtypes.bfloat16))       # [64, 2]
    in_maps = []
    for core in range(8):
        b, h = divmod(core, 2)
        xb = x[b]                                    # [N, F]
        xh = xb[h * HALF:(h + 1) * HALF]             # [HALF, F]
        xfp = np.ascontiguousarray(
            xb.reshape(NCH, 128, F).transpose(1, 0, 2).reshape(128, NCH * F))
        xhp = np.ascontiguousarray(
            xh.reshape(NIC, 128, F).transpose(1, 0, 2).reshape(128, NIC * F))
        in_maps.append({
            "xfp": xfp,
            "xhp": xhp,
            "xht": np.ascontiguousarray(xh.T.astype(ml_dtypes.bfloat16)),  # [64, HALF]
            "wqkr": wqkr,
            "wqkf": wqkf,
        })
    return in_maps


def assemble(results):
    out = np.empty((BZ, F, N), dtype=np.float32)
    for core in range(8):
        b, h = divmod(core, 2)
        o = results[core]["out"].reshape(128, NIC, F)        # [p, ic, f]
        # i_local = ic*128 + p  ->  [f, ic, p] then flatten
        out[b, :, h * HALF:(h + 1) * HALF] = o.transpose(2, 1, 0).reshape(F, HALF)
    return out


def kernel(x, weight_key, weight_query, _trace=False, _tmpdir=None):
    from concourse.bass_utils import run_bass_kernel_spmd

    nc = build_program()
    in_maps = make_in_maps(x, weight_key, weight_query)
    res = run_bass_kernel_spmd(nc, in_maps, core_ids=list(range(8)), trace=_trace,
                               tmpdir=_tmpdir)
    out = assemble(res.results)
    if _trace:
        return out, res
    return out
